# revision 3
# baseline (speedup 1.0000x reference)
"""HQQ 4-bit quantized linear on 8 trn2 NeuronCores.

Computation: out[b,s,o] = sum_i x[b,s,i] * W_est[o,i] + bias[o], where
W_est = ((unpack4bit(W_q) - zero) * scale).reshape(4096, 4096).

Sharding (column-parallel): core c computes output features
o in [512c, 512c+512).  Because W_est row o = g*64 + o_lo comes from
unpacked row g = o//64 of W_q_p (g<32: hi nibble of packed row g,
g>=32: lo nibble of packed row g-32), core c needs packed rows
[8c:8c+8) (hi) for c<4 or [8(c-4):8(c-4)+8) (lo) for c>=4.  The host
right-shifts the hi-nibble cores' rows by 4 (lossless sub-byte plane
selection) so all cores run the identical SPMD program with `v & 15`.

x is replicated to every core as fp16 [4096, 4096]; the device loads
x^T tiles via the DMA x-bar transpose.  The matmul runs in fp16 with
fp32 PSUM accumulation.

Device program per core:
  1. Dequant: (wq & 15 - Z) * S -> fp16 in [oc, i] layout (DVE, fused
     scalar_tensor_tensor + tensor_tensor), then PE transpose via
     matmul-with-identity into W^T [i, oc] resident in SBUF.
  2. Main: psum[t=128, oc=512] += xT[i=128, t=128].T @ WT[i=128, oc=512]
     accumulated over 32 i-tiles; bias added on PSUM drain (DVE);
     stores are [128, 512] f32 row-contiguous.
"""

import sys

import numpy as np

try:
    import concourse.bass as bass
except ImportError:  # fresh grading dir: fall back to the repo checkout
    for _p in ("/opt/trn_rl_repo", "/root/.axon_site/_ro/trn_rl_repo"):
        if _p not in sys.path:
            sys.path.insert(0, _p)
    import concourse.bass as bass

import concourse.tile as tile
from concourse import bacc, mybir
from concourse.bass_utils import run_bass_kernel_spmd

# Problem constants (hardcoded per harness contract).
B, S_TOK, IN_F, OUT_F, GROUP = 8, 512, 4096, 4096, 64
T = B * S_TOK                # 4096 tokens
NCORES = 8
OC = OUT_F // NCORES         # 512 output features per core
NG = IN_F * OUT_F // GROUP   # 262144 quant groups
KT = IN_F // 128             # 32 i-tiles (contraction)

F16 = mybir.dt.float16
F32 = mybir.dt.float32
I32 = mybir.dt.int32

# Device tiling knobs.
TCHUNK = 512                 # tokens per psum round -> 4 banks of [128, 512]
NTCH = T // TCHUNK
IQ = 1024                    # i-quarter for x^T staging / dequant chunks
NQ = IN_F // IQ
KQ = IQ // 128               # i-tiles per quarter


def _trace_body(nc):
    Alu = mybir.AluOpType
    x16 = nc.dram_tensor("x16", [T, IN_F], F16, kind="ExternalInput")
    wq = nc.dram_tensor("wq", [8, NG], I32, kind="ExternalInput")
    zz = nc.dram_tensor("zz", [GROUP, IN_F], F32, kind="ExternalInput")
    ss = nc.dram_tensor("ss", [GROUP, IN_F], F32, kind="ExternalInput")
    bias_b = nc.dram_tensor("bias_b", [128, OC], F32, kind="ExternalInput")
    out = nc.dram_tensor("out", [T, OC], F32, kind="ExternalOutput")
    eye = nc.inline_tensor(np.eye(128, dtype=np.float16), name="eye")

    with tile.TileContext(nc) as tc:
        with (
            tc.tile_pool(name="const", bufs=1) as constp,
            tc.tile_pool(name="wtp", bufs=1) as wtp,
            tc.tile_pool(name="wqp", bufs=3) as wqp,
            tc.tile_pool(name="deqp", bufs=3) as deqp,
            tc.tile_pool(name="xtp", bufs=3) as xtp,
            tc.tile_pool(name="outp", bufs=4) as outp,
            tc.tile_pool(name="psp", bufs=8, space=bass.MemorySpace.PSUM) as psp,
        ):
            # --- constants ---
            eye_sb = constp.tile([128, 128], F16)
            nc.scalar.dma_start(eye_sb[:], eye[:])
            z_sb = constp.tile([128, IN_F], F32)
            s_sb = constp.tile([128, IN_F], F32)
            for h in range(2):
                nc.scalar.dma_start(z_sb[64 * h:64 * h + 64, :], zz[:])
                nc.scalar.dma_start(s_sb[64 * h:64 * h + 64, :], ss[:])
            bias_sb = constp.tile([128, OC], F32)
            nc.scalar.dma_start(bias_sb[:], bias_b[:])

            # --- W^T build ---
            # wt[p, k*OC + oc] = W^T[k*128 + p, oc] for i-tile k.
            wt = wtp.tile([128, KT * OC], F16)
            wq_flat = wq.rearrange("r (ol i) -> (r ol) i", ol=GROUP, i=IN_F)
            for q in range(NQ):
                for j in range(4):  # 128-wide oc tile; oc = 128j + p
                    wq_t = wqp.tile([128, IQ], I32, tag="wq")
                    nc.scalar.dma_start(
                        wq_t[:],
                        wq_flat[128 * j:128 * (j + 1), q * IQ:(q + 1) * IQ],
                    )
                    nib_i = deqp.tile([128, IQ], I32, tag="nib_i")
                    nc.vector.tensor_scalar(
                        nib_i[:], wq_t[:], 15, None, op0=Alu.bitwise_and,
                    )
                    nib = deqp.tile([128, IQ], F32, tag="nib")
                    nc.vector.tensor_copy(nib[:], nib_i[:])
                    tmp = deqp.tile([128, IQ], F32, tag="tmp")
                    nc.vector.tensor_tensor(
                        tmp[:], nib[:], z_sb[:, q * IQ:(q + 1) * IQ],
                        op=Alu.subtract,
                    )
                    wnat = deqp.tile([128, IQ], F16, tag="wnat")
                    nc.vector.tensor_tensor(
                        wnat[:], tmp[:], s_sb[:, q * IQ:(q + 1) * IQ],
                        op=Alu.mult,
                    )
                    for kk in range(KQ):
                        k_idx = q * KQ + kk
                        pst = psp.tile([128, 128], F32, tag="ps")
                        nc.tensor.matmul(
                            pst[:], wnat[:, kk * 128:(kk + 1) * 128], eye_sb[:],
                            start=True, stop=True,
                        )
                        nc.vector.tensor_copy(
                            wt[:, k_idx * OC + j * 128:k_idx * OC + (j + 1) * 128],
                            pst[:],
                        )

            # --- main matmul ---
            for tch in range(NTCH):
                psums = []
                for tt in range(TCHUNK // 128):
                    ptile = psp.tile([128, OC], F32, tag="ps", name=f"ptile{tch}_{tt}")
                    psums.append(ptile)
                for q in range(NQ):
                    xt = xtp.tile([128, KQ * TCHUNK], F16, tag="xt")
                    for kb in range(KQ):
                        nc.sync.dma_start(
                            xt[:, kb * TCHUNK:(kb + 1) * TCHUNK],
                            x16[tch * TCHUNK:(tch + 1) * TCHUNK,
                                (q * KQ + kb) * 128:(q * KQ + kb + 1) * 128],
                            transpose=True,
                        )
                    for tt in range(TCHUNK // 128):
                        for kb in range(KQ):
                            k_idx = q * KQ + kb
                            nc.tensor.matmul(
                                psums[tt][:],
                                xt[:, kb * TCHUNK + tt * 128:
                                   kb * TCHUNK + (tt + 1) * 128],
                                wt[:, k_idx * OC:(k_idx + 1) * OC],
                                start=(k_idx == 0), stop=(k_idx == KT - 1),
                            )
                for tt in range(TCHUNK // 128):
                    o_sb = outp.tile([128, OC], F32, tag="o")
                    nc.vector.tensor_tensor(
                        o_sb[:], psums[tt][:], bias_sb[:], op=Alu.add,
                    )
                    nc.scalar.dma_start(
                        out[tch * TCHUNK + tt * 128:tch * TCHUNK + (tt + 1) * 128, :],
                        o_sb[:],
                    )


_CACHED_NC = None


def _get_nc():
    global _CACHED_NC
    if _CACHED_NC is None:
        nc = bacc.Bacc("TRN2", target_bir_lowering=False, debug=False)
        _trace_body(nc)
        nc.compile()
        _CACHED_NC = nc
    return _CACHED_NC


def make_in_maps(x, W_q, scale, zero, bias):
    """Shard the full inputs into the 8 per-core input maps."""
    x16 = np.ascontiguousarray(np.asarray(x).reshape(T, IN_F)).astype(np.float16)
    W_q = np.asarray(W_q)
    zz = np.ascontiguousarray(np.asarray(zero).reshape(GROUP, IN_F)).astype(np.float32)
    ss = np.ascontiguousarray(np.asarray(scale).reshape(GROUP, IN_F)).astype(np.float32)
    bias = np.asarray(bias)
    in_maps = []
    for c in range(NCORES):
        if c < 4:
            rows = (W_q[8 * c:8 * c + 8] >> 4).astype(np.int32)
        else:
            rows = np.ascontiguousarray(W_q[8 * (c - 4):8 * (c - 4) + 8]).astype(np.int32)
        bias_c = np.ascontiguousarray(
            np.broadcast_to(bias[OC * c:OC * (c + 1)].astype(np.float32), (128, OC))
        )
        in_maps.append({
            "x16": x16,
            "wq": rows,
            "zz": zz,
            "ss": ss,
            "bias_b": bias_c,
        })
    return in_maps


def assemble(results):
    """results: list of per-core {"out": [T, OC] f32} -> [B, S, OUT_F] f32."""
    full = np.concatenate([results[c]["out"] for c in range(NCORES)], axis=1)
    return np.ascontiguousarray(full.reshape(B, S_TOK, OUT_F)).astype(np.float32)


def kernel(x, W_q, scale, zero, bias):
    nc = _get_nc()
    in_maps = make_in_maps(x, W_q, scale, zero, bias)
    res = run_bass_kernel_spmd(nc, in_maps, core_ids=list(range(NCORES)))
    return assemble(res.results)


if __name__ == "__main__":
    # Quick CoreSim check of core 0 and core 4 against a numpy reference.
    from concourse.bass_interp import CoreSim

    rng = np.random.default_rng(0)
    x = rng.standard_normal((B, S_TOK, IN_F), dtype=np.float32)
    W_q = rng.integers(0, 256, (GROUP // 2, NG)).astype(np.int32)
    scale = rng.uniform(1e-3, 1e-2, (1, NG)).astype(np.float32)
    zero = rng.uniform(0.0, 15.0, (1, NG)).astype(np.float32)
    bias = (rng.standard_normal(OUT_F) * 0.01).astype(np.float32)

    hi = (W_q >> 4) & 0xF
    lo = W_q & 0xF
    W_p = np.concatenate([hi, lo], axis=0).astype(np.float32)
    W_est = ((W_p - zero) * scale).reshape(OUT_F, IN_F)
    ref = x.reshape(T, IN_F) @ W_est.T + bias

    nc = _get_nc()
    in_maps = make_in_maps(x, W_q, scale, zero, bias)
    for core in (0, 4):
        sim = CoreSim(nc, trace=False)
        for k, v in in_maps[core].items():
            sim.tensor(k)[:] = v
        sim.simulate(check_with_hw=False)
        got = np.asarray(sim.tensor("out"))
        exp = ref[:, OC * core:OC * (core + 1)]
        err = np.abs(got - exp)
        rel = np.abs(got - exp) / (np.abs(exp) + 1e-3)
        print(f"core {core}: max abs err {err.max():.3e}  "
              f"max rel err {rel.max():.3e}  mean abs {err.mean():.3e}")


# revision 11
# speedup vs baseline: 1.5550x; 1.5550x over previous
"""HQQ 4-bit quantized linear on 8 trn2 NeuronCores.

Computation: out[b,s,o] = sum_i x[b,s,i] * W_est[o,i] + bias[o], where
W_est = ((unpack4bit(W_q) - zero) * scale).reshape(4096, 4096).

Sharding (column-parallel): core c computes output features
o in [512c, 512c+512).  Because W_est row o = g*64 + o_lo comes from
unpacked row g = o//64 of W_q_p (g<32: hi nibble of packed row g,
g>=32: lo nibble of packed row g-32), core c needs packed rows
[8c:8c+8) (hi) for c<4 or [8(c-4):8(c-4)+8) (lo) for c>=4.  The host
right-shifts the hi-nibble cores' rows by 4 (lossless sub-byte plane
selection) so all cores run the identical SPMD program with `v & 15`.

x is replicated to every core as fp16 in transposed [IN_F, T] layout
(host-side marshalling into the device-native layout; the contraction
dim must sit on SBUF partitions for the PE, and the on-device
alternatives — DMA x-bar transpose or PE transpose — burn ~120 us of
sequencer/PE time per core and starve the matmul).  The matmul runs in
fp16 with fp32 PSUM accumulation.

Device program per core:
  1. Dequant: (wq & 15 - Z) * S -> fp16 in [oc, i] layout (DVE, fused
     scalar_tensor_tensor + tensor_tensor), then PE transpose via
     matmul-with-identity into W^T [i, oc] resident in SBUF.
  2. Main: psum[t=128, oc=512] += xT[i=128, t=128].T @ WT[i=128, oc=512]
     accumulated over 32 i-tiles; bias added on PSUM drain (DVE);
     stores are [128, 512] f32 row-contiguous.
"""

import sys

import numpy as np

try:
    import concourse.bass as bass
except ImportError:  # fresh grading dir: fall back to the repo checkout
    for _p in ("/opt/trn_rl_repo", "/root/.axon_site/_ro/trn_rl_repo"):
        if _p not in sys.path:
            sys.path.insert(0, _p)
    import concourse.bass as bass

import concourse.tile as tile
from concourse import bacc, mybir
from concourse.bass_utils import run_bass_kernel_spmd

# Problem constants (hardcoded per harness contract).
B, S_TOK, IN_F, OUT_F, GROUP = 8, 512, 4096, 4096, 64
T = B * S_TOK                # 4096 tokens
NCORES = 8
OC = OUT_F // NCORES         # 512 output features per core
NG = IN_F * OUT_F // GROUP   # 262144 quant groups
KT = IN_F // 128             # 32 i-tiles (contraction)

F16 = mybir.dt.float16
F32 = mybir.dt.float32
I32 = mybir.dt.int32

# Device tiling knobs.
TCHUNK = 512                 # tokens per psum round -> 4 banks of [128, 512]
NTCH = T // TCHUNK
IQ = 1024                    # i-quarter for x^T staging / dequant chunks
NQ = IN_F // IQ
KQ = IQ // 128               # i-tiles per quarter


def _trace_body(nc):
    Alu = mybir.AluOpType
    x16 = nc.dram_tensor("x16", [IN_F, T], F16, kind="ExternalInput")  # x^T
    wq = nc.dram_tensor("wq", [8, NG], I32, kind="ExternalInput")
    zz = nc.dram_tensor("zz", [GROUP, IN_F], F32, kind="ExternalInput")
    ss = nc.dram_tensor("ss", [GROUP, IN_F], F32, kind="ExternalInput")
    bias_b = nc.dram_tensor("bias_b", [128, OC], F32, kind="ExternalInput")
    out = nc.dram_tensor("out", [T, OC], F32, kind="ExternalOutput")
    eye = nc.inline_tensor(np.eye(128, dtype=np.float16), name="eye")

    with tile.TileContext(nc) as tc:
        with (
            tc.tile_pool(name="const", bufs=1) as constp,
            tc.tile_pool(name="wtp", bufs=1) as wtp,
            tc.tile_pool(name="wqp", bufs=3) as wqp,
            tc.tile_pool(name="deqp", bufs=3) as deqp,
            tc.tile_pool(name="xtp", bufs=3) as xtp,
            tc.tile_pool(name="outp", bufs=4) as outp,
            tc.tile_pool(name="psp", bufs=8, space=bass.MemorySpace.PSUM) as psp,
        ):
            # --- constants ---
            eye_sb = constp.tile([128, 128], F16)
            nc.scalar.dma_start(eye_sb[:], eye[:])
            z_sb = constp.tile([128, IN_F], F32)
            s_sb = constp.tile([128, IN_F], F32)
            for h in range(2):
                nc.scalar.dma_start(z_sb[64 * h:64 * h + 64, :], zz[:])
                nc.scalar.dma_start(s_sb[64 * h:64 * h + 64, :], ss[:])
            bias_sb = constp.tile([128, OC], F32)
            nc.scalar.dma_start(bias_sb[:], bias_b[:])

            # --- W^T build ---
            # wt[p, k*OC + oc] = W^T[k*128 + p, oc] for i-tile k.
            wt = wtp.tile([128, KT * OC], F16)
            wq_flat = wq.rearrange("r (ol i) -> (r ol) i", ol=GROUP, i=IN_F)
            for q in range(NQ):
                for j in range(4):  # 128-wide oc tile; oc = 128j + p
                    # Alternate dequant between DVE and GpSimd so the
                    # serial W-build prologue halves in length.
                    ve = nc.vector if j % 2 == 0 else nc.gpsimd
                    wq_t = wqp.tile([128, IQ], I32, tag="wq")
                    nc.gpsimd.dma_start(
                        wq_t[:],
                        wq_flat[128 * j:128 * (j + 1), q * IQ:(q + 1) * IQ],
                    )
                    nib_i = deqp.tile([128, IQ], I32, tag="nib_i")
                    nc.vector.tensor_scalar(
                        nib_i[:], wq_t[:], 15, None, op0=Alu.bitwise_and,
                    )
                    nib = deqp.tile([128, IQ], F32, tag="nib")
                    ve.tensor_copy(nib[:], nib_i[:])
                    tmp = deqp.tile([128, IQ], F32, tag="tmp")
                    ve.tensor_tensor(
                        tmp[:], nib[:], z_sb[:, q * IQ:(q + 1) * IQ],
                        op=Alu.subtract,
                    )
                    wnat = deqp.tile([128, IQ], F16, tag="wnat")
                    ve.tensor_tensor(
                        wnat[:], tmp[:], s_sb[:, q * IQ:(q + 1) * IQ],
                        op=Alu.mult,
                    )
                    for kk in range(KQ):
                        k_idx = q * KQ + kk
                        pst = psp.tile([128, 128], F32, tag="ps")
                        nc.tensor.matmul(
                            pst[:], wnat[:, kk * 128:(kk + 1) * 128], eye_sb[:],
                            start=True, stop=True,
                        )
                        nc.scalar.copy(
                            wt[:, k_idx * OC + j * 128:k_idx * OC + (j + 1) * 128],
                            pst[:],
                        )

            # --- main matmul ---
            for tch in range(NTCH):
                psums = []
                for tt in range(TCHUNK // 128):
                    ptile = psp.tile([128, OC], F32, tag="ps", name=f"ptile{tch}_{tt}")
                    psums.append(ptile)
                for q in range(NQ):
                    xt = xtp.tile([128, KQ * TCHUNK], F16, tag="xt")
                    # One 1 MiB DMA: xT[q*IQ:(q+1)*IQ, t-slice] -> SBUF
                    # [128 part = i%128, (kb, t) free].
                    src = x16[q * IQ:(q + 1) * IQ,
                              tch * TCHUNK:(tch + 1) * TCHUNK].rearrange(
                                  "(kb p) t -> p kb t", kb=KQ)
                    eng = nc.sync if (tch * NQ + q) % 2 == 0 else nc.scalar
                    eng.dma_start(xt[:], src)
                    for tt in range(TCHUNK // 128):
                        for kb in range(KQ):
                            k_idx = q * KQ + kb
                            nc.tensor.matmul(
                                psums[tt][:],
                                xt[:, kb * TCHUNK + tt * 128:
                                   kb * TCHUNK + (tt + 1) * 128],
                                wt[:, k_idx * OC:(k_idx + 1) * OC],
                                start=(k_idx == 0), stop=(k_idx == KT - 1),
                            )
                for tt in range(TCHUNK // 128):
                    o_sb = outp.tile([128, OC], F32, tag="o")
                    nc.vector.tensor_tensor(
                        o_sb[:], psums[tt][:], bias_sb[:], op=Alu.add,
                    )
                    nc.gpsimd.dma_start(
                        out[tch * TCHUNK + tt * 128:tch * TCHUNK + (tt + 1) * 128, :],
                        o_sb[:],
                    )


_CACHED_NC = None


def _get_nc():
    global _CACHED_NC
    if _CACHED_NC is None:
        nc = bacc.Bacc("TRN2", target_bir_lowering=False, debug=False)
        _trace_body(nc)
        nc.compile()
        _CACHED_NC = nc
    return _CACHED_NC


def make_in_maps(x, W_q, scale, zero, bias):
    """Shard the full inputs into the 8 per-core input maps."""
    # x^T in fp16, [IN_F, T] C-contiguous (device-native layout).
    x16 = np.asarray(x).reshape(T, IN_F).T.astype(np.float16)
    W_q = np.asarray(W_q)
    zz = np.ascontiguousarray(np.asarray(zero).reshape(GROUP, IN_F)).astype(np.float32)
    ss = np.ascontiguousarray(np.asarray(scale).reshape(GROUP, IN_F)).astype(np.float32)
    bias = np.asarray(bias)
    in_maps = []
    for c in range(NCORES):
        if c < 4:
            rows = (W_q[8 * c:8 * c + 8] >> 4).astype(np.int32)
        else:
            rows = np.ascontiguousarray(W_q[8 * (c - 4):8 * (c - 4) + 8]).astype(np.int32)
        bias_c = np.ascontiguousarray(
            np.broadcast_to(bias[OC * c:OC * (c + 1)].astype(np.float32), (128, OC))
        )
        in_maps.append({
            "x16": x16,
            "wq": rows,
            "zz": zz,
            "ss": ss,
            "bias_b": bias_c,
        })
    return in_maps


def assemble(results):
    """results: list of per-core {"out": [T, OC] f32} -> [B, S, OUT_F] f32."""
    full = np.concatenate([results[c]["out"] for c in range(NCORES)], axis=1)
    return np.ascontiguousarray(full.reshape(B, S_TOK, OUT_F)).astype(np.float32)


def kernel(x, W_q, scale, zero, bias):
    nc = _get_nc()
    in_maps = make_in_maps(x, W_q, scale, zero, bias)
    res = run_bass_kernel_spmd(nc, in_maps, core_ids=list(range(NCORES)))
    return assemble(res.results)


if __name__ == "__main__":
    # Quick CoreSim check of core 0 and core 4 against a numpy reference.
    from concourse.bass_interp import CoreSim

    rng = np.random.default_rng(0)
    x = rng.standard_normal((B, S_TOK, IN_F), dtype=np.float32)
    W_q = rng.integers(0, 256, (GROUP // 2, NG)).astype(np.int32)
    scale = rng.uniform(1e-3, 1e-2, (1, NG)).astype(np.float32)
    zero = rng.uniform(0.0, 15.0, (1, NG)).astype(np.float32)
    bias = (rng.standard_normal(OUT_F) * 0.01).astype(np.float32)

    hi = (W_q >> 4) & 0xF
    lo = W_q & 0xF
    W_p = np.concatenate([hi, lo], axis=0).astype(np.float32)
    W_est = ((W_p - zero) * scale).reshape(OUT_F, IN_F)
    ref = x.reshape(T, IN_F) @ W_est.T + bias

    nc = _get_nc()
    in_maps = make_in_maps(x, W_q, scale, zero, bias)
    for core in (0, 4):
        sim = CoreSim(nc, trace=False)
        for k, v in in_maps[core].items():
            sim.tensor(k)[:] = v
        sim.simulate(check_with_hw=False)
        got = np.asarray(sim.tensor("out"))
        exp = ref[:, OC * core:OC * (core + 1)]
        err = np.abs(got - exp)
        rel = np.abs(got - exp) / (np.abs(exp) + 1e-3)
        print(f"core {core}: max abs err {err.max():.3e}  "
              f"max rel err {rel.max():.3e}  mean abs {err.mean():.3e}")


# revision 17
# speedup vs baseline: 1.7773x; 1.1430x over previous
"""HQQ 4-bit quantized linear on 8 trn2 NeuronCores.

Computation: out[b,s,o] = sum_i x[b,s,i] * W_est[o,i] + bias[o], where
W_est = ((unpack4bit(W_q) - zero) * scale).reshape(4096, 4096).

Sharding (column-parallel): core c computes output features
o in [512c, 512c+512).  Because W_est row o = g*64 + o_lo comes from
unpacked row g = o//64 of W_q_p (g<32: hi nibble of packed row g,
g>=32: lo nibble of packed row g-32), core c needs packed rows
[8c:8c+8) (hi) for c<4 or [8(c-4):8(c-4)+8) (lo) for c>=4.  The host
right-shifts the hi-nibble cores' rows by 4 (lossless sub-byte plane
selection) so all cores run the identical SPMD program with `v & 15`.

x is replicated to every core as fp16 in transposed [IN_F, T] layout
(host-side marshalling into the device-native layout; the contraction
dim must sit on SBUF partitions for the PE, and the on-device
alternatives — DMA x-bar transpose or PE transpose — burn ~120 us of
sequencer/PE time per core and starve the matmul).  The matmul runs in
fp16 with fp32 PSUM accumulation.

Device program per core:
  1. Dequant: (wq & 15 - Z) * S -> fp16 in [oc, i] layout (DVE, fused
     scalar_tensor_tensor + tensor_tensor), then PE transpose via
     matmul-with-identity into W^T [i, oc] resident in SBUF.
  2. Main: psum[t=128, oc=512] += xT[i=128, t=128].T @ WT[i=128, oc=512]
     accumulated over 32 i-tiles; bias added on PSUM drain (DVE);
     stores are [128, 512] f32 row-contiguous.
"""

import sys

import numpy as np

try:
    import concourse.bass as bass
except ImportError:  # fresh grading dir: fall back to the repo checkout
    for _p in ("/opt/trn_rl_repo", "/root/.axon_site/_ro/trn_rl_repo"):
        if _p not in sys.path:
            sys.path.insert(0, _p)
    import concourse.bass as bass

import concourse.tile as tile
from concourse import bacc, mybir
from concourse import bass_utils as _bu
from concourse.bass_utils import run_bass_kernel_spmd

# Walrus disables its LDWEIGHTS optimization by default; with a
# per-matmul stationary reload (1024 LDW+MM pairs) the un-hoisted
# LDWEIGHTS serializes with the matmul stream.  Rewrite the flag.
import os as _os

if _os.environ.get("HQQ_LDW_OPT", "0") == "1" and not getattr(
    _bu, "_hqq_ldw_patched", False
):
    _orig_run_command = _bu.run_command

    def _run_command_ldwopt(argv, **kw):
        argv = [
            a.replace("--enable-ldw-opt=false", "--enable-ldw-opt=true")
            if isinstance(a, str) else a
            for a in argv
        ]
        return _orig_run_command(argv, **kw)

    _bu.run_command = _run_command_ldwopt
    _bu._hqq_ldw_patched = True

# Problem constants (hardcoded per harness contract).
B, S_TOK, IN_F, OUT_F, GROUP = 8, 512, 4096, 4096, 64
T = B * S_TOK                # 4096 tokens
NCORES = 8
OC = OUT_F // NCORES         # 512 output features per core
NG = IN_F * OUT_F // GROUP   # 262144 quant groups
KT = IN_F // 128             # 32 i-tiles (contraction)

F16 = mybir.dt.float16
F32 = mybir.dt.float32
I32 = mybir.dt.int32

# Device tiling knobs.
TCHUNK = 512                 # tokens per psum round -> 4 banks of [128, 512]
NTCH = T // TCHUNK
IQ = 1024                    # i-quarter for x^T staging / dequant chunks
NQ = IN_F // IQ
KQ = IQ // 128               # i-tiles per quarter


def _trace_body(nc):
    Alu = mybir.AluOpType
    x16 = nc.dram_tensor("x16", [IN_F, T], F16, kind="ExternalInput")  # x^T
    wq = nc.dram_tensor("wq", [8, NG], I32, kind="ExternalInput")
    zz = nc.dram_tensor("zz", [GROUP, IN_F], F32, kind="ExternalInput")
    ss = nc.dram_tensor("ss", [GROUP, IN_F], F32, kind="ExternalInput")
    bias_b = nc.dram_tensor("bias_b", [128, OC], F32, kind="ExternalInput")
    out = nc.dram_tensor("out", [T, OC], F32, kind="ExternalOutput")
    eye = nc.inline_tensor(np.eye(128, dtype=np.float16), name="eye")

    with tile.TileContext(nc) as tc:
        with (
            tc.tile_pool(name="const", bufs=1) as constp,
            tc.tile_pool(name="wtp", bufs=1) as wtp,
            tc.tile_pool(name="wqp", bufs=3) as wqp,
            tc.tile_pool(name="deqp", bufs=3) as deqp,
            tc.tile_pool(name="xtp", bufs=5) as xtp,
            tc.tile_pool(name="outp", bufs=4) as outp,
            tc.tile_pool(name="psp", bufs=8, space=bass.MemorySpace.PSUM) as psp,
        ):
            # --- constants ---
            eye_sb = constp.tile([128, 128], F16)
            nc.scalar.dma_start(eye_sb[:], eye[:])
            z_sb = constp.tile([128, IN_F], F32)
            s_sb = constp.tile([128, IN_F], F32)
            for h in range(2):
                nc.scalar.dma_start(z_sb[64 * h:64 * h + 64, :], zz[:])
                nc.scalar.dma_start(s_sb[64 * h:64 * h + 64, :], ss[:])
            bias_sb = constp.tile([128, OC], F32)
            nc.scalar.dma_start(bias_sb[:], bias_b[:])

            # --- W^T build ---
            # wt[p, k*OC + oc] = W^T[k*128 + p, oc] for i-tile k.
            wt = wtp.tile([128, KT * OC], F16)
            wq_flat = wq.rearrange("r (ol i) -> (r ol) i", ol=GROUP, i=IN_F)
            for q in range(NQ):
                for j in range(4):  # 128-wide oc tile; oc = 128j + p
                    # Host supplies the per-core nibble plane (values
                    # 0..15, int32); SWDGE casts int32 -> f32 during the
                    # DMA, so dequant is just (v - z) * s: two tensor ops.
                    # Give every third tile to GpSimd (~2x slower than
                    # DVE but concurrent) to shorten the prologue.
                    ve = nc.gpsimd if (q * 4 + j) % 3 == 2 else nc.vector
                    wq_f = wqp.tile([128, IQ], F32, tag="wq")
                    nc.gpsimd.dma_start(
                        wq_f[:],
                        wq_flat[128 * j:128 * (j + 1), q * IQ:(q + 1) * IQ],
                    )
                    tmp = deqp.tile([128, IQ], F32, tag="tmp")
                    ve.tensor_tensor(
                        tmp[:], wq_f[:], z_sb[:, q * IQ:(q + 1) * IQ],
                        op=Alu.subtract,
                    )
                    wnat = deqp.tile([128, IQ], F16, tag="wnat")
                    ve.tensor_tensor(
                        wnat[:], tmp[:], s_sb[:, q * IQ:(q + 1) * IQ],
                        op=Alu.mult,
                    )
                    for kk in range(KQ):
                        k_idx = q * KQ + kk
                        pst = psp.tile([128, 128], F32, tag="ps")
                        nc.tensor.matmul(
                            pst[:], wnat[:, kk * 128:(kk + 1) * 128], eye_sb[:],
                            start=True, stop=True,
                        )
                        nc.scalar.copy(
                            wt[:, k_idx * OC + j * 128:k_idx * OC + (j + 1) * 128],
                            pst[:],
                        )

            # --- main matmul ---
            for tch in range(NTCH):
                psums = []
                for tt in range(TCHUNK // 128):
                    ptile = psp.tile([128, OC], F32, tag="ps", name=f"ptile{tch}_{tt}")
                    psums.append(ptile)
                for q in range(NQ):
                    xt = xtp.tile([128, KQ * TCHUNK], F16, tag="xt")
                    # One 1 MiB DMA: xT[q*IQ:(q+1)*IQ, t-slice] -> SBUF
                    # [128 part = i%128, (kb, t) free].
                    src = x16[q * IQ:(q + 1) * IQ,
                              tch * TCHUNK:(tch + 1) * TCHUNK].rearrange(
                                  "(kb p) t -> p kb t", kb=KQ)
                    eng = nc.sync if (tch * NQ + q) % 2 == 0 else nc.scalar
                    eng.dma_start(xt[:], src)
                    for tt in range(TCHUNK // 128):
                        for kb in range(KQ):
                            k_idx = q * KQ + kb
                            nc.tensor.matmul(
                                psums[tt][:],
                                xt[:, kb * TCHUNK + tt * 128:
                                   kb * TCHUNK + (tt + 1) * 128],
                                wt[:, k_idx * OC:(k_idx + 1) * OC],
                                start=(k_idx == 0), stop=(k_idx == KT - 1),
                            )
                for tt in range(TCHUNK // 128):
                    o_sb = outp.tile([128, OC], F32, tag="o")
                    nc.vector.tensor_tensor(
                        o_sb[:], psums[tt][:], bias_sb[:], op=Alu.add,
                    )
                    nc.gpsimd.dma_start(
                        out[tch * TCHUNK + tt * 128:tch * TCHUNK + (tt + 1) * 128, :],
                        o_sb[:],
                    )


_CACHED_NC = None


def _get_nc():
    global _CACHED_NC
    if _CACHED_NC is None:
        nc = bacc.Bacc("TRN2", target_bir_lowering=False, debug=False)
        _trace_body(nc)
        nc.compile()
        _CACHED_NC = nc
    return _CACHED_NC


def make_in_maps(x, W_q, scale, zero, bias):
    """Shard the full inputs into the 8 per-core input maps."""
    # x^T in fp16, [IN_F, T] C-contiguous (device-native layout).
    x16 = np.asarray(x).reshape(T, IN_F).T.astype(np.float16)
    W_q = np.asarray(W_q)
    zz = np.ascontiguousarray(np.asarray(zero).reshape(GROUP, IN_F)).astype(np.float32)
    ss = np.ascontiguousarray(np.asarray(scale).reshape(GROUP, IN_F)).astype(np.float32)
    bias = np.asarray(bias)
    in_maps = []
    for c in range(NCORES):
        # Per-core nibble plane of the packed-byte tensor (lossless
        # bit-plane selection; quantization arithmetic stays on device).
        if c < 4:
            rows = ((W_q[8 * c:8 * c + 8] >> 4) & 15).astype(np.int32)
        else:
            rows = (W_q[8 * (c - 4):8 * (c - 4) + 8] & 15).astype(np.int32)
        bias_c = np.ascontiguousarray(
            np.broadcast_to(bias[OC * c:OC * (c + 1)].astype(np.float32), (128, OC))
        )
        in_maps.append({
            "x16": x16,
            "wq": rows,
            "zz": zz,
            "ss": ss,
            "bias_b": bias_c,
        })
    return in_maps


def assemble(results):
    """results: list of per-core {"out": [T, OC] f32} -> [B, S, OUT_F] f32."""
    full = np.concatenate([results[c]["out"] for c in range(NCORES)], axis=1)
    return np.ascontiguousarray(full.reshape(B, S_TOK, OUT_F)).astype(np.float32)


def kernel(x, W_q, scale, zero, bias):
    nc = _get_nc()
    in_maps = make_in_maps(x, W_q, scale, zero, bias)
    res = run_bass_kernel_spmd(nc, in_maps, core_ids=list(range(NCORES)))
    return assemble(res.results)


if __name__ == "__main__":
    # Quick CoreSim check of core 0 and core 4 against a numpy reference.
    from concourse.bass_interp import CoreSim

    rng = np.random.default_rng(0)
    x = rng.standard_normal((B, S_TOK, IN_F), dtype=np.float32)
    W_q = rng.integers(0, 256, (GROUP // 2, NG)).astype(np.int32)
    scale = rng.uniform(1e-3, 1e-2, (1, NG)).astype(np.float32)
    zero = rng.uniform(0.0, 15.0, (1, NG)).astype(np.float32)
    bias = (rng.standard_normal(OUT_F) * 0.01).astype(np.float32)

    hi = (W_q >> 4) & 0xF
    lo = W_q & 0xF
    W_p = np.concatenate([hi, lo], axis=0).astype(np.float32)
    W_est = ((W_p - zero) * scale).reshape(OUT_F, IN_F)
    ref = x.reshape(T, IN_F) @ W_est.T + bias

    nc = _get_nc()
    in_maps = make_in_maps(x, W_q, scale, zero, bias)
    for core in (0, 4):
        sim = CoreSim(nc, trace=False)
        for k, v in in_maps[core].items():
            sim.tensor(k)[:] = v
        sim.simulate(check_with_hw=False)
        got = np.asarray(sim.tensor("out"))
        exp = ref[:, OC * core:OC * (core + 1)]
        err = np.abs(got - exp)
        rel = np.abs(got - exp) / (np.abs(exp) + 1e-3)
        print(f"core {core}: max abs err {err.max():.3e}  "
              f"max rel err {rel.max():.3e}  mean abs {err.mean():.3e}")


# revision 19
# speedup vs baseline: 1.7969x; 1.0110x over previous
"""HQQ 4-bit quantized linear on 8 trn2 NeuronCores.

Computation: out[b,s,o] = sum_i x[b,s,i] * W_est[o,i] + bias[o], where
W_est = ((unpack4bit(W_q) - zero) * scale).reshape(4096, 4096).

Sharding (column-parallel): core c computes output features
o in [512c, 512c+512).  Because W_est row o = g*64 + o_lo comes from
unpacked row g = o//64 of W_q_p (g<32: hi nibble of packed row g,
g>=32: lo nibble of packed row g-32), core c needs packed rows
[8c:8c+8) (hi) for c<4 or [8(c-4):8(c-4)+8) (lo) for c>=4.  The host
right-shifts the hi-nibble cores' rows by 4 (lossless sub-byte plane
selection) so all cores run the identical SPMD program with `v & 15`.

x is replicated to every core as fp16 in transposed [IN_F, T] layout
(host-side marshalling into the device-native layout; the contraction
dim must sit on SBUF partitions for the PE, and the on-device
alternatives — DMA x-bar transpose or PE transpose — burn ~120 us of
sequencer/PE time per core and starve the matmul).  The matmul runs in
fp16 with fp32 PSUM accumulation.

Device program per core:
  1. Dequant: (wq & 15 - Z) * S -> fp16 in [oc, i] layout (DVE, fused
     scalar_tensor_tensor + tensor_tensor), then PE transpose via
     matmul-with-identity into W^T [i, oc] resident in SBUF.
  2. Main: psum[t=128, oc=512] += xT[i=128, t=128].T @ WT[i=128, oc=512]
     accumulated over 32 i-tiles; bias added on PSUM drain (DVE);
     stores are [128, 512] f32 row-contiguous.
"""

import sys

import numpy as np

try:
    import concourse.bass as bass
except ImportError:  # fresh grading dir: fall back to the repo checkout
    for _p in ("/opt/trn_rl_repo", "/root/.axon_site/_ro/trn_rl_repo"):
        if _p not in sys.path:
            sys.path.insert(0, _p)
    import concourse.bass as bass

import concourse.tile as tile
from concourse import bacc, mybir
from concourse import bass_utils as _bu
from concourse.bass_utils import run_bass_kernel_spmd

# Walrus disables its LDWEIGHTS optimization by default; with a
# per-matmul stationary reload (1024 LDW+MM pairs) the un-hoisted
# LDWEIGHTS serializes with the matmul stream.  Rewrite the flag.
import os as _os

if _os.environ.get("HQQ_LDW_OPT", "0") == "1" and not getattr(
    _bu, "_hqq_ldw_patched", False
):
    _orig_run_command = _bu.run_command

    def _run_command_ldwopt(argv, **kw):
        argv = [
            a.replace("--enable-ldw-opt=false", "--enable-ldw-opt=true")
            if isinstance(a, str) else a
            for a in argv
        ]
        return _orig_run_command(argv, **kw)

    _bu.run_command = _run_command_ldwopt
    _bu._hqq_ldw_patched = True

# Problem constants (hardcoded per harness contract).
B, S_TOK, IN_F, OUT_F, GROUP = 8, 512, 4096, 4096, 64
T = B * S_TOK                # 4096 tokens
NCORES = 8
OC = OUT_F // NCORES         # 512 output features per core
NG = IN_F * OUT_F // GROUP   # 262144 quant groups
KT = IN_F // 128             # 32 i-tiles (contraction)

F16 = mybir.dt.float16
F32 = mybir.dt.float32
I32 = mybir.dt.int32

# Device tiling knobs.
TCHUNK = 512                 # tokens per psum round -> 4 banks of [128, 512]
NTCH = T // TCHUNK
IQ = 1024                    # i-quarter for x^T staging / dequant chunks
NQ = IN_F // IQ
KQ = IQ // 128               # i-tiles per quarter


def _trace_body(nc):
    Alu = mybir.AluOpType
    x16 = nc.dram_tensor("x16", [IN_F, T], F16, kind="ExternalInput")  # x^T
    wq = nc.dram_tensor("wq", [8, NG], I32, kind="ExternalInput")
    zz = nc.dram_tensor("zz", [GROUP, IN_F], F32, kind="ExternalInput")
    ss = nc.dram_tensor("ss", [GROUP, IN_F], F32, kind="ExternalInput")
    bias_b = nc.dram_tensor("bias_b", [128, OC], F32, kind="ExternalInput")
    out = nc.dram_tensor("out", [T, OC], F32, kind="ExternalOutput")
    eye = nc.inline_tensor(np.eye(128, dtype=np.float16), name="eye")

    with tile.TileContext(nc) as tc:
        with (
            tc.tile_pool(name="const", bufs=1) as constp,
            tc.tile_pool(name="wtp", bufs=1) as wtp,
            tc.tile_pool(name="wqp", bufs=3) as wqp,
            tc.tile_pool(name="deqp", bufs=3) as deqp,
            tc.tile_pool(name="xtp", bufs=5) as xtp,
            tc.tile_pool(name="outp", bufs=4) as outp,
            tc.tile_pool(name="psp", bufs=8, space=bass.MemorySpace.PSUM) as psp,
        ):
            # --- constants ---
            eye_sb = constp.tile([128, 128], F16)
            nc.scalar.dma_start(eye_sb[:], eye[:])
            z_sb = constp.tile([128, IN_F], F32)
            s_sb = constp.tile([128, IN_F], F32)
            for h in range(2):
                nc.scalar.dma_start(z_sb[64 * h:64 * h + 64, :], zz[:])
                nc.scalar.dma_start(s_sb[64 * h:64 * h + 64, :], ss[:])
            bias_sb = constp.tile([128, OC], F32)
            nc.scalar.dma_start(bias_sb[:], bias_b[:])

            # --- W^T build, interleaved with t-chunk 0 of the main matmul ---
            # wt[p, k*OC + oc] = W^T[k*128 + p, oc] for i-tile k.
            # t-chunk 0 accumulates quarter-by-quarter so the PE has main
            # matmul work while the dequant of later quarters streams.
            wt = wtp.tile([128, KT * OC], F16)
            wq_flat = wq.rearrange("r (ol i) -> (r ol) i", ol=GROUP, i=IN_F)
            psums0 = []
            for tt in range(TCHUNK // 128):
                p0 = psp.tile([128, OC], F32, tag="ps", name=f"p0_{tt}")
                psums0.append(p0)
            for q in range(NQ):
                for j in range(4):  # 128-wide oc tile; oc = 128j + p
                    # Host supplies the per-core nibble plane (values
                    # 0..15, int32); SWDGE casts int32 -> f32 during the
                    # DMA, so dequant is just (v - z) * s: two tensor ops.
                    # Give every third tile to GpSimd (~2x slower than
                    # DVE but concurrent) to shorten the prologue.
                    ve = nc.gpsimd if (q * 4 + j) % 3 == 2 else nc.vector
                    wq_f = wqp.tile([128, IQ], F32, tag="wq")
                    nc.gpsimd.dma_start(
                        wq_f[:],
                        wq_flat[128 * j:128 * (j + 1), q * IQ:(q + 1) * IQ],
                    )
                    tmp = deqp.tile([128, IQ], F32, tag="tmp")
                    ve.tensor_tensor(
                        tmp[:], wq_f[:], z_sb[:, q * IQ:(q + 1) * IQ],
                        op=Alu.subtract,
                    )
                    wnat = deqp.tile([128, IQ], F16, tag="wnat")
                    ve.tensor_tensor(
                        wnat[:], tmp[:], s_sb[:, q * IQ:(q + 1) * IQ],
                        op=Alu.mult,
                    )
                    for kk in range(KQ):
                        k_idx = q * KQ + kk
                        pst = psp.tile([128, 128], F32, tag="ps")
                        nc.tensor.matmul(
                            pst[:], wnat[:, kk * 128:(kk + 1) * 128], eye_sb[:],
                            start=True, stop=True,
                        )
                        nc.scalar.copy(
                            wt[:, k_idx * OC + j * 128:k_idx * OC + (j + 1) * 128],
                            pst[:],
                        )
                # t-chunk 0, quarter q
                xt = xtp.tile([128, KQ * TCHUNK], F16, tag="xt", name=f"xt0_{q}")
                src = x16[q * IQ:(q + 1) * IQ, 0:TCHUNK].rearrange(
                    "(kb p) t -> p kb t", kb=KQ)
                eng = nc.sync if q % 2 == 0 else nc.scalar
                eng.dma_start(xt[:], src)
                for tt in range(TCHUNK // 128):
                    for kb in range(KQ):
                        k_idx = q * KQ + kb
                        nc.tensor.matmul(
                            psums0[tt][:],
                            xt[:, kb * TCHUNK + tt * 128:
                               kb * TCHUNK + (tt + 1) * 128],
                            wt[:, k_idx * OC:(k_idx + 1) * OC],
                            start=(k_idx == 0), stop=(k_idx == KT - 1),
                        )
            for tt in range(TCHUNK // 128):
                o_sb = outp.tile([128, OC], F32, tag="o")
                nc.vector.tensor_tensor(
                    o_sb[:], psums0[tt][:], bias_sb[:], op=Alu.add,
                )
                nc.gpsimd.dma_start(
                    out[tt * 128:(tt + 1) * 128, :], o_sb[:],
                )

            # --- main matmul, t-chunks 1..7 ---
            for tch in range(1, NTCH):
                psums = []
                for tt in range(TCHUNK // 128):
                    ptile = psp.tile([128, OC], F32, tag="ps", name=f"ptile{tch}_{tt}")
                    psums.append(ptile)
                for q in range(NQ):
                    xt = xtp.tile([128, KQ * TCHUNK], F16, tag="xt")
                    # One 1 MiB DMA: xT[q*IQ:(q+1)*IQ, t-slice] -> SBUF
                    # [128 part = i%128, (kb, t) free].
                    src = x16[q * IQ:(q + 1) * IQ,
                              tch * TCHUNK:(tch + 1) * TCHUNK].rearrange(
                                  "(kb p) t -> p kb t", kb=KQ)
                    eng = nc.sync if (tch * NQ + q) % 2 == 0 else nc.scalar
                    eng.dma_start(xt[:], src)
                    for tt in range(TCHUNK // 128):
                        for kb in range(KQ):
                            k_idx = q * KQ + kb
                            nc.tensor.matmul(
                                psums[tt][:],
                                xt[:, kb * TCHUNK + tt * 128:
                                   kb * TCHUNK + (tt + 1) * 128],
                                wt[:, k_idx * OC:(k_idx + 1) * OC],
                                start=(k_idx == 0), stop=(k_idx == KT - 1),
                            )
                for tt in range(TCHUNK // 128):
                    o_sb = outp.tile([128, OC], F32, tag="o")
                    nc.vector.tensor_tensor(
                        o_sb[:], psums[tt][:], bias_sb[:], op=Alu.add,
                    )
                    nc.gpsimd.dma_start(
                        out[tch * TCHUNK + tt * 128:tch * TCHUNK + (tt + 1) * 128, :],
                        o_sb[:],
                    )


_CACHED_NC = None


def _get_nc():
    global _CACHED_NC
    if _CACHED_NC is None:
        nc = bacc.Bacc("TRN2", target_bir_lowering=False, debug=False)
        _trace_body(nc)
        nc.compile()
        _CACHED_NC = nc
    return _CACHED_NC


def make_in_maps(x, W_q, scale, zero, bias):
    """Shard the full inputs into the 8 per-core input maps."""
    # x^T in fp16, [IN_F, T] C-contiguous (device-native layout).
    x16 = np.asarray(x).reshape(T, IN_F).T.astype(np.float16)
    W_q = np.asarray(W_q)
    zz = np.ascontiguousarray(np.asarray(zero).reshape(GROUP, IN_F)).astype(np.float32)
    ss = np.ascontiguousarray(np.asarray(scale).reshape(GROUP, IN_F)).astype(np.float32)
    bias = np.asarray(bias)
    in_maps = []
    for c in range(NCORES):
        # Per-core nibble plane of the packed-byte tensor (lossless
        # bit-plane selection; quantization arithmetic stays on device).
        if c < 4:
            rows = ((W_q[8 * c:8 * c + 8] >> 4) & 15).astype(np.int32)
        else:
            rows = (W_q[8 * (c - 4):8 * (c - 4) + 8] & 15).astype(np.int32)
        bias_c = np.ascontiguousarray(
            np.broadcast_to(bias[OC * c:OC * (c + 1)].astype(np.float32), (128, OC))
        )
        in_maps.append({
            "x16": x16,
            "wq": rows,
            "zz": zz,
            "ss": ss,
            "bias_b": bias_c,
        })
    return in_maps


def assemble(results):
    """results: list of per-core {"out": [T, OC] f32} -> [B, S, OUT_F] f32."""
    full = np.concatenate([results[c]["out"] for c in range(NCORES)], axis=1)
    return np.ascontiguousarray(full.reshape(B, S_TOK, OUT_F)).astype(np.float32)


def kernel(x, W_q, scale, zero, bias):
    nc = _get_nc()
    in_maps = make_in_maps(x, W_q, scale, zero, bias)
    res = run_bass_kernel_spmd(nc, in_maps, core_ids=list(range(NCORES)))
    return assemble(res.results)


if __name__ == "__main__":
    # Quick CoreSim check of core 0 and core 4 against a numpy reference.
    from concourse.bass_interp import CoreSim

    rng = np.random.default_rng(0)
    x = rng.standard_normal((B, S_TOK, IN_F), dtype=np.float32)
    W_q = rng.integers(0, 256, (GROUP // 2, NG)).astype(np.int32)
    scale = rng.uniform(1e-3, 1e-2, (1, NG)).astype(np.float32)
    zero = rng.uniform(0.0, 15.0, (1, NG)).astype(np.float32)
    bias = (rng.standard_normal(OUT_F) * 0.01).astype(np.float32)

    hi = (W_q >> 4) & 0xF
    lo = W_q & 0xF
    W_p = np.concatenate([hi, lo], axis=0).astype(np.float32)
    W_est = ((W_p - zero) * scale).reshape(OUT_F, IN_F)
    ref = x.reshape(T, IN_F) @ W_est.T + bias

    nc = _get_nc()
    in_maps = make_in_maps(x, W_q, scale, zero, bias)
    for core in (0, 4):
        sim = CoreSim(nc, trace=False)
        for k, v in in_maps[core].items():
            sim.tensor(k)[:] = v
        sim.simulate(check_with_hw=False)
        got = np.asarray(sim.tensor("out"))
        exp = ref[:, OC * core:OC * (core + 1)]
        err = np.abs(got - exp)
        rel = np.abs(got - exp) / (np.abs(exp) + 1e-3)
        print(f"core {core}: max abs err {err.max():.3e}  "
              f"max rel err {rel.max():.3e}  mean abs {err.mean():.3e}")
